# revision 29
# baseline (speedup 1.0000x reference)
"""Trainium2 Bass kernel for nn_Attention_46840913330813.

Full attention layer: QKV proj + partial RoPE (rot=20 of 80) + causal
softmax attention + output proj.  B=2, S=2048, H=2560, 32 heads x 80.

Sharding: tensor-parallel over heads for QKV+attention (4 heads/core on
8 cores); AllToAll collectives then redistribute the per-head attention
outputs so the output projection is row-parallel (each core owns 128
query rows per batch-half and multiplies by the FULL Wo).  Four
quarter-size AllToAlls (one per batch half, ~0.7 MB each) are launched
as soon as their half of the attention finishes, so all but the last
hide under compute.  All matmul operands bf16 (tolerance 2e-2), PSUM
accumulation f32.

Schedule (single PSUM scope; 2 banks QKV proj, 4 banks scores, 2 banks
attn-accum/out-proj; Q/K seq->d transposes are plain identity matmuls
into drained psum regions — they pipeline at ~N/f marginal cost unlike
transpose-mode or DMA-XBAR):
  W1: phase A batch 0 (chunk 0 runs kt-outer over 8 psum accumulators
      so the PE tracks the cold-start DMA front; QKV proj + RoPE +
      transpose, all kept in SBUF)
  W2: phase A batch 1 interleaved with phase B batch 0
      A2A(b0,h0) after the 8th iter, A2A(b0,h1) at the end
  W3: phase B batch 1 with phase C batch 0 pieces from iter 6 on;
      A2A(b1,h1) at the end, its ag load issued from the gpsimd queue
      (a waiting DMA on sync/scalar would block the ring in-order)
  tail: C(b0) cc2..4 cover the last collective, then C(b1).
Ring discipline: sync carries a_sb outputs (collective inputs) + bulk
loads; scalar carries tables/x_hi/odd w slices + the wt2 prefetch; exp
activations share the scalar engine so no waiting DMA may precede them.
Host reassembles 128-row slices.
"""

import math

import numpy as np
import ml_dtypes

import concourse.mybir as mybir
import concourse.tile as tile
from concourse import bacc
from concourse.bass_utils import run_bass_kernel_spmd

N_CORES = 8
B, S, H = 2, 2048, 2560
BS = B * S                      # 4096
NH, HD = 32, 80                 # heads, head dim
HL = NH // N_CORES              # 4 local heads
DL = HL * HD                    # 320 local feature width
ROT = 20                        # rotary dims
THETA = 10000.0
KT = H // 128                   # 20 contraction tiles
SCALE = 1.0 / math.sqrt(HD)
SHIFT = -5.0                    # uniform pre-exp shift (cancels in softmax)
QCH = 512                       # attention q-chunk
NQC = S // QCH                  # 4 q chunks per batch
ACH = 512                       # phase A chunk width
NCPB = S // ACH                 # 4 phase A chunks per batch
RPH = 128                      # output rows per core per batch-half
CCH = 512                       # phase C output-column chunk
NCC = H // CCH                  # 5 phase C column chunks

F32 = mybir.dt.float32
BF16 = mybir.dt.bfloat16

_cache = {}


def build_bass():
    nc = bacc.Bacc(None, target_bir_lowering=False, debug=False,
                   num_devices=N_CORES)

    # Single packed input (one dispatch arg): columns are
    #   [ xT (BS) | wall (3*DL) | wo (H) | cosP 640 | sinP 640 | idtri 256 ]
    # xT/wall/wo span all H rows; the tables live in rows 0:128 with the
    # per-128-row-tile layout the kernel loads (cosP[p, m*20+d]).
    PKW = BS + 3 * DL + H + 640 + 640 + 256
    C_W, C_WO, C_CS = BS, BS + 3 * DL, BS + 3 * DL + H
    pack = nc.declare_dram_parameter("pack", [H, PKW], BF16, isOutput=False)
    xT = pack[:, 0:BS]
    wall = pack[:, C_W:C_W + 3 * DL]
    wo = pack[:, C_WO:C_WO + H]
    out = nc.declare_dram_parameter("out", [B * 2 * RPH, H], F32,
                                    isOutput=True)

    with tile.TileContext(nc) as tc:
        with tc.tile_pool(name="dram", bufs=1, space="DRAM") as dram:
            a2a_in = [[dram.tile([N_CORES * DL, RPH], BF16,
                                 name=f"a2a_in{b}{hf}", tag=f"a2a_in{b}{hf}")
                       for hf in range(2)] for b in range(B)]
            a2a_out = [[dram.tile([N_CORES * DL, RPH], BF16,
                                  name=f"a2a_out{b}{hf}",
                                  tag=f"a2a_out{b}{hf}")
                        for hf in range(2)] for b in range(B)]

            with tc.tile_pool(name="persist", bufs=1) as persist, \
                 tc.tile_pool(name="mpool", bufs=1) as mpool, \
                 tc.tile_pool(name="epool", bufs=8) as epool, \
                 tc.tile_pool(name="apool", bufs=4) as apool, \
                 tc.tile_pool(name="nat_ps", bufs=2, space="PSUM") as nat_ps, \
                 tc.tile_pool(name="big_ps", bufs=2, space="PSUM") as big_ps, \
                 tc.tile_pool(name="sm_ps", bufs=2, space="PSUM") as sm_ps:

                # QT/KT per batch: [d, t(q/k), h, seq]; V ones-augmented.
                # Partition dim 128: rows 80:128 hold transpose padding
                # garbage that phase B never reads.
                stage = [persist.tile([128, 2, HL, S], BF16, name=f"stage{b}",
                                      tag=f"stage{b}") for b in range(B)]
                v_st = [persist.tile([128, S // 128, HL, HD + 1], BF16,
                                     name=f"v{b}", tag=f"v{b}")
                        for b in range(B)]

                idtri = mpool.tile([128, 256], BF16, name="idtri")
                shift_sb = mpool.tile([128, 1], F32, name="shift_sb")
                onesA = mpool.tile([128, 1], F32, name="onesA")

                def emit_collective(b, hf):
                    nc.gpsimd.collective_compute(
                        "AllToAll", mybir.AluOpType.bypass,
                        replica_groups=[list(range(N_CORES))],
                        ins=[a2a_in[b][hf][:]], outs=[a2a_out[b][hf][:]])

                def emit_B_iter(b, qc, h):
                    """Attention for one (batch, 512-q chunk, head)."""
                    q0 = qc * QCH
                    qap = stage[b][0:HD, 0, h, q0:q0 + QCH]
                    aps_t = sm_ps.tile([128, QCH], F32, name="aps",
                                       tag="small")
                    aps = aps_t[0:HD + 1, :]
                    nfull = qc * (QCH // 128)
                    # off-diagonal k-tiles, exp'd in pairs
                    for kp in range(nfull // 2):
                        sps = big_ps.tile([128, 2 * QCH], F32, name="sps",
                                          tag="big")
                        ex = epool.tile([128, 2 * QCH], BF16, name="ex",
                                        tag="exp")
                        for g in range(2):
                            kt = 2 * kp + g
                            nc.tensor.matmul(
                                sps[:, g * QCH:(g + 1) * QCH],
                                stage[b][0:HD, 1, h, kt * 128:(kt + 1) * 128],
                                qap, start=True, stop=True)
                        nc.scalar.activation(
                            ex[:], sps[:], mybir.ActivationFunctionType.Exp,
                            bias=shift_sb[:], scale=SCALE)
                        for g in range(2):
                            kt = 2 * kp + g
                            nc.tensor.matmul(
                                aps[:], v_st[b][:, kt, h, :],
                                ex[:, g * QCH:(g + 1) * QCH],
                                start=(kt == 0), stop=False)
                    # diagonal tiles o=0..3 at widths 512/384/256/128,
                    # grouped (0,1) and (2,3); one 128x128 triangle mask
                    for (o1, o2) in ((0, 1), (2, 3)):
                        w1, w2 = QCH - o1 * 128, QCH - o2 * 128
                        sps = big_ps.tile([128, 2 * QCH], F32, name="sps",
                                          tag="big")
                        ex = epool.tile([128, 2 * QCH], BF16, name="ex",
                                        tag="exp")
                        for (o, w, c0) in ((o1, w1, 0), (o2, w2, w1)):
                            kt = nfull + o
                            nc.tensor.matmul(
                                sps[:, c0:c0 + w],
                                stage[b][0:HD, 1, h, kt * 128:(kt + 1) * 128],
                                stage[b][0:HD, 0, h, q0 + o * 128:q0 + QCH],
                                start=True, stop=True)
                        nc.scalar.activation(
                            ex[:, 0:w1 + w2], sps[:, 0:w1 + w2],
                            mybir.ActivationFunctionType.Exp,
                            bias=shift_sb[:], scale=SCALE)
                        for (o, w, c0) in ((o1, w1, 0), (o2, w2, w1)):
                            nc.vector.tensor_mul(ex[:, c0:c0 + 128],
                                                 ex[:, c0:c0 + 128],
                                                 idtri[:, 128:256])
                            kt = nfull + o
                            nc.tensor.matmul(
                                aps[:, o * 128:QCH], v_st[b][:, kt, h, :],
                                ex[:, c0:c0 + w],
                                start=(kt == 0), stop=(o == 3))
                    rec = apool.tile([1, QCH], F32, name="rec", tag="rec")
                    nc.vector.reciprocal_approx_fast(rec[:], aps[0:1, :])
                    rb = apool.tile([HD + 1, QCH], F32, name="rb", tag="rb")
                    nc.gpsimd.partition_broadcast(rb[:], rec[:])
                    a_sb = apool.tile([HD + 1, QCH], BF16, name="a_sb",
                                      tag="a_out")
                    nc.vector.tensor_mul(a_sb[:], aps[:], rb[:])
                    hf = qc // 2
                    blk0 = (qc % 2) * 4
                    dst = a2a_in[b][hf].rearrange(
                        "(blk d) n -> d blk n", blk=8)[
                        h * HD:(h + 1) * HD, blk0:blk0 + 4, :]
                    nc.sync.dma_start(
                        dst, a_sb[1:HD + 1, :].rearrange(
                            "d (q n) -> d q n", n=RPH))

                def emit_C_piece(b, hf, cc, wt, ag):
                    """out[(b,hf) 128 rows, cc-th 512 cols]."""
                    cps_t = sm_ps.tile([128, QCH], F32, name="cps",
                                       tag="small")
                    cps = cps_t[:, 0:CCH]
                    for ft in range(KT):
                        nc.tensor.matmul(cps[:], ag[:, ft, :], wt[:, ft, :],
                                         start=(ft == 0), stop=(ft == KT - 1))
                    o_sb = opool.tile([128, CCH], F32, name="o_sb",
                                      tag="o_sb")
                    nc.vector.tensor_copy(o_sb[:], cps[:])
                    r0 = (b * 2 + hf) * RPH
                    nc.sync.dma_start(
                        out[r0:r0 + RPH, cc * CCH:(cc + 1) * CCH], o_sb[:])

                def load_ag(b, hf):
                    # sync ring: a waiting DMA on the Act ring would block
                    # the exp activations queued behind it
                    ag = agpool.tile([128, KT, RPH], BF16, name="ag",
                                     tag="ag")
                    nc.sync.dma_start(
                        ag[:], a2a_out[b][hf].rearrange("(t p) n -> p t n",
                                                        p=128))
                    return ag

                def load_wo(cc, eng=None):
                    wt = wopool.tile([128, KT, CCH], BF16, name="wt",
                                     tag="wo")
                    (eng or nc.sync).dma_start(
                        wt[:], wo.rearrange("(t p) n -> p t n",
                                            p=128)[:, :,
                                                   cc * CCH:(cc + 1) * CCH])
                    return wt

                with tc.tile_pool(name="wpool", bufs=1) as wpool, \
                     tc.tile_pool(name="xpool", bufs=4) as xpool, \
                     tc.tile_pool(name="cpool", bufs=1) as cpool, \
                     tc.tile_pool(name="sbA", bufs=3) as sbA:

                    HK = KT // 2

                    def load_x(b, ci):
                        # split head pieces so the chunk's first matmuls can
                        # start after ~0.25MB instead of the full 2.5MB
                        g0 = b * S + ci * ACH
                        csl = slice(g0, g0 + ACH)
                        lo = xT[0:HK * 128, csl].rearrange(
                            "(t p) n -> p t n", p=128)
                        hi = xT[HK * 128:H, csl].rearrange(
                            "(t p) n -> p t n", p=128)
                        x_lo = xpool.tile([128, HK, ACH], BF16, name="x_lo",
                                          tag="x")
                        nc.sync.dma_start(x_lo[:, 0:2, :], lo[:, 0:2, :])
                        nc.sync.dma_start(x_lo[:, 2:HK, :], lo[:, 2:HK, :])
                        x_hi = xpool.tile([128, HK, ACH], BF16, name="x_hi",
                                          tag="x")
                        nc.scalar.dma_start(x_hi[:, 0:2, :], hi[:, 0:2, :])
                        nc.scalar.dma_start(x_hi[:, 2:HK, :], hi[:, 2:HK, :])
                        return x_lo, x_hi

                    # DMA front, interleaved by need-time of the first pass:
                    # sync:   xlo[0:2] w0 xlo[2:10] w2 w4 .. w18
                    # scalar: idtri cos sin xhi[0:2] w1 w3 xhi[2:10] w5..w19
                    w_sb = wpool.tile([128, KT, 3 * DL], BF16, name="w_sb")
                    wview = wall.rearrange("(t p) n -> p t n", p=128)
                    lo0 = xT[0:HK * 128, 0:ACH].rearrange(
                        "(t p) n -> p t n", p=128)
                    hi0 = xT[HK * 128:H, 0:ACH].rearrange(
                        "(t p) n -> p t n", p=128)
                    x_lo0 = xpool.tile([128, HK, ACH], BF16, name="x_lo",
                                       tag="x")
                    x_hi0 = xpool.tile([128, HK, ACH], BF16, name="x_hi",
                                       tag="x")
                    nc.sync.dma_start(x_lo0[:, 0:2, :], lo0[:, 0:2, :])
                    nc.scalar.dma_start(idtri[:],
                                        pack[0:128, C_CS + 1280:C_CS + 1536])
                    cosN_sb = cpool.tile([128, BS // 128, ROT], BF16,
                                         name="cosN_sb")
                    nc.scalar.dma_start(
                        cosN_sb[:],
                        pack[0:128, C_CS:C_CS + 640].rearrange(
                            "p (m d) -> p m d", d=ROT))
                    sinN_sb = cpool.tile([128, BS // 128, ROT], BF16,
                                         name="sinN_sb")
                    nc.scalar.dma_start(
                        sinN_sb[:],
                        pack[0:128, C_CS + 640:C_CS + 1280].rearrange(
                            "p (m d) -> p m d", d=ROT))
                    nc.sync.dma_start(w_sb[:, 0, :], wview[:, 0, :])
                    nc.scalar.dma_start(x_hi0[:, 0:2, :], hi0[:, 0:2, :])
                    nc.scalar.dma_start(w_sb[:, 1, :], wview[:, 1, :])
                    nc.scalar.dma_start(w_sb[:, 3, :], wview[:, 3, :])
                    nc.sync.dma_start(x_lo0[:, 2:HK, :], lo0[:, 2:HK, :])
                    for kt in range(2, KT, 2):
                        nc.sync.dma_start(w_sb[:, kt, :], wview[:, kt, :])
                    nc.scalar.dma_start(x_hi0[:, 2:HK, :], hi0[:, 2:HK, :])
                    for kt in range(5, KT, 2):
                        nc.scalar.dma_start(w_sb[:, kt, :], wview[:, kt, :])
                    x0 = (x_lo0, x_hi0)
                    # throwaway matmuls keep the PE busy through the DMA
                    # front so the p-state ramp finishes before real work
                    warm = cpool.tile([128, 16], BF16, name="warm")
                    nc.vector.memset(warm[:], 0.0)
                    wps = nat_ps.tile([128, QCH], F32, name="wps", tag="nat")
                    for i in range(110):
                        nc.tensor.matmul(wps[0:16, 0:16], warm[:], warm[:],
                                         start=(i == 0), stop=(i == 109))
                    nc.vector.memset(shift_sb[:], SHIFT)
                    nc.vector.memset(onesA[:], 1.0)
                    for b in range(B):
                        nc.vector.tensor_copy(
                            v_st[b][:, :, :, 0:1],
                            onesA[:, :, None, None].to_broadcast(
                                (128, S // 128, HL, 1)))

                    def drain_mt(b, mtl, mtg, psA, psB, tps):
                        """psum -> qk_sb -> rope -> v_st/stage for one
                        128-row tile.  psA/psB are [128, 480] psum APs;
                        tps = (tpA, tpB) [HD, 512] psum APs for the
                        identity-matmul transposes."""
                        qk_sb = sbA.tile([128, 2, HL, 128], BF16,
                                         name="qk_sb", tag="qk")
                        nc.vector.tensor_copy(
                            qk_sb[:, 0, :, 0:HD],
                            psA[:, 0:4 * HD].rearrange(
                                "p (h d) -> p h d", d=HD))
                        nc.vector.tensor_copy(
                            qk_sb[:, 1, 0:2, 0:HD],
                            psA[:, 4 * HD:480].rearrange(
                                "p (h d) -> p h d", d=HD))
                        nc.vector.tensor_copy(
                            qk_sb[:, 1, 2:4, 0:HD],
                            psB[:, 0:2 * HD].rearrange(
                                "p (h d) -> p h d", d=HD))
                        # rope: q' = q*cos + swap(q)*sin_signed
                        rtmp = sbA.tile([128, 2, HL, ROT], BF16,
                                        name="rtmp", tag="rt")
                        half = ROT // 2
                        cosb = cosN_sb[:, mtg, None, None, :].to_broadcast(
                            (128, 2, HL, ROT))
                        sinb = sinN_sb[:, mtg, None, None, :].to_broadcast(
                            (128, 2, HL, ROT))
                        nc.vector.tensor_mul(rtmp[:, :, :, 0:half],
                                             qk_sb[:, :, :, half:ROT],
                                             sinb[:, :, :, 0:half])
                        nc.vector.tensor_mul(rtmp[:, :, :, half:ROT],
                                             qk_sb[:, :, :, 0:half],
                                             sinb[:, :, :, half:ROT])
                        nc.vector.tensor_mul(qk_sb[:, :, :, 0:ROT],
                                             qk_sb[:, :, :, 0:ROT], cosb)
                        nc.vector.tensor_add(qk_sb[:, :, :, 0:ROT],
                                             qk_sb[:, :, :, 0:ROT],
                                             rtmp[:])
                        nc.vector.tensor_copy(
                            v_st[b][:, mtl, :, 1:HD + 1],
                            psB[:, 160:480].rearrange(
                                "p (h d) -> p h d", h=HL))
                        # transposes as plain identity matmuls: they
                        # pipeline with the projection matmuls (~70ns
                        # marginal) unlike transpose-mode (~275ns flat)
                        for t in range(2):
                            tp = tps[t]
                            for h in range(HL):
                                nc.tensor.matmul(
                                    tp[:, h * 128:(h + 1) * 128],
                                    qk_sb[:, t, h, 0:HD], idtri[:, 0:128],
                                    start=True, stop=True)
                            nc.vector.tensor_copy(
                                stage[b][0:HD, t, :,
                                         mtl * 128:(mtl + 1) * 128],
                                tp[:].rearrange("d (h n) -> d h n",
                                                h=HL))

                    def emit_A_chunk(b, ci, xpre=None):
                        """QKV projection for one 512-row chunk."""
                        x_lo, x_hi = xpre if xpre else load_x(b, ci)

                        def xk(kt):
                            return (x_lo[:, kt, :] if kt < HK
                                    else x_hi[:, kt - HK, :])

                        for mt in range(ACH // 128):
                            mtl = ci * (ACH // 128) + mt     # tile in batch
                            mtg = b * (S // 128) + mtl       # global tile
                            # two 1-bank psum passes (480 cols each) keep
                            # phase A's psum rotation decoupled from B's
                            psA = nat_ps.tile([128, QCH], F32, name="psA",
                                              tag="nat")
                            for kt in range(KT):
                                nc.tensor.matmul(
                                    psA[:, 0:480],
                                    xk(kt)[:, mt * 128:(mt + 1) * 128],
                                    w_sb[:, kt, 0:480],
                                    start=(kt == 0), stop=(kt == KT - 1))
                            psB = nat_ps.tile([128, QCH], F32, name="psB",
                                              tag="nat")
                            for kt in range(KT):
                                nc.tensor.matmul(
                                    psB[:, 0:480],
                                    xk(kt)[:, mt * 128:(mt + 1) * 128],
                                    w_sb[:, kt, 480:960],
                                    start=(kt == 0), stop=(kt == KT - 1))
                            tp_t = big_ps.tile([128, 2 * QCH], F32,
                                               name="tp", tag="big")
                            drain_mt(b, mtl, mtg, psA[:, 0:480],
                                     psB[:, 0:480],
                                     (tp_t[0:HD, 0:QCH],
                                      tp_t[0:HD, QCH:2 * QCH]))

                    def emit_A_chunk0(x_lo, x_hi):
                        """Chunk (0,0): kt-outer over 8 psum accumulators so
                        the PE consumes each k-tile at DMA arrival pace
                        during the cold-start front."""
                        def xk(kt):
                            return (x_lo[:, kt, :] if kt < HK
                                    else x_hi[:, kt - HK, :])

                        big1 = big_ps.tile([128, 2 * QCH], F32, name="c0a",
                                           tag="big")
                        big2 = big_ps.tile([128, 2 * QCH], F32, name="c0b",
                                           tag="big")
                        nat1 = nat_ps.tile([128, QCH], F32, name="c0c",
                                           tag="nat")
                        nat2 = nat_ps.tile([128, QCH], F32, name="c0d",
                                           tag="nat")
                        sm1 = sm_ps.tile([128, QCH], F32, name="c0e",
                                         tag="small")
                        sm2 = sm_ps.tile([128, QCH], F32, name="c0f",
                                         tag="small")
                        psA = [big1[:, 0:480], big1[:, QCH:QCH + 480],
                               big2[:, 0:480], big2[:, QCH:QCH + 480]]
                        psB = [nat1[:, 0:480], nat2[:, 0:480],
                               sm1[:, 0:480], sm2[:, 0:480]]
                        for kt in range(KT):
                            for mt in range(4):
                                nc.tensor.matmul(
                                    psA[mt],
                                    xk(kt)[:, mt * 128:(mt + 1) * 128],
                                    w_sb[:, kt, 0:480],
                                    start=(kt == 0), stop=(kt == KT - 1))
                            for mt in range(4):
                                nc.tensor.matmul(
                                    psB[mt],
                                    xk(kt)[:, mt * 128:(mt + 1) * 128],
                                    w_sb[:, kt, 480:960],
                                    start=(kt == 0), stop=(kt == KT - 1))
                        # tp reuses each tile's own drained banks
                        tps = [(big1[0:HD, 0:QCH], nat1[0:HD, 0:QCH]),
                               (big1[0:HD, QCH:2 * QCH], nat2[0:HD, 0:QCH]),
                               (big2[0:HD, 0:QCH], sm1[0:HD, 0:QCH]),
                               (big2[0:HD, QCH:2 * QCH], sm2[0:HD, 0:QCH])]
                        for mt in range(4):
                            drain_mt(0, mt, mt, psA[mt], psB[mt], tps[mt])

                    # W1: phase A batch 0
                    emit_A_chunk0(*x0)
                    for ci in range(1, NCPB):
                        emit_A_chunk(0, ci)
                    # W2: phase A batch 1 interleaved with phase B batch 0
                    # AND the already-enabled phase B batch 1 blocks (block
                    # qc only needs A(b1) chunks 0..qc).  B blocks go BEFORE
                    # their paired A chunk so the Act engine drains the exps
                    # during the projection matmuls.
                    for ci in range(NCPB):
                        for h in range(HL):
                            emit_B_iter(0, ci, h)
                        if ci == 1:
                            emit_collective(0, 0)
                        if ci == 3:
                            emit_collective(0, 1)
                        if ci >= 1:
                            for h in range(HL):
                                emit_B_iter(1, ci - 1, h)
                            if ci == 2:
                                emit_collective(1, 0)
                        emit_A_chunk(1, ci)

                # W3: the remaining phase B batch 1 block (qc=3), then C.
                # Each wo slice is loaded ONCE (cc-outer loop over all four
                # (b,hf) groups); ag tiles prefetched right after their
                # collectives land.
                with tc.tile_pool(name="wopool", bufs=3) as wopool, \
                     tc.tile_pool(name="agpool", bufs=4) as agpool, \
                     tc.tile_pool(name="opool", bufs=2) as opool:
                    ag0, ag1 = load_ag(0, 0), load_ag(0, 1)
                    bg0 = load_ag(1, 0)         # a2a(1,0) landed in W2
                    # wt2 prefetched on the idle scalar ring so cc=2 starts
                    # right after the B loop (sync stays clear for a_sb)
                    wt2 = load_wo(2, eng=nc.scalar)
                    for h in range(HL):
                        emit_B_iter(1, 3, h)
                        if h >= 2:
                            wt = load_wo(h - 2)
                            emit_C_piece(0, 0, h - 2, wt, ag0)
                            emit_C_piece(0, 1, h - 2, wt, ag1)
                    emit_collective(1, 1)
                    # bg1 issues from the gpsimd queue right after its
                    # collective so the sync ring never blocks on the wait
                    bg1 = agpool.tile([128, KT, RPH], BF16, name="ag",
                                      tag="ag")
                    nc.gpsimd.dma_start(
                        bg1[:], a2a_out[1][1].rearrange("(t p) n -> p t n",
                                                        p=128))
                    # batch-0 pieces first (their deps are long satisfied);
                    # the batch-1 group comes last so its hoisted PE-side
                    # wait on bg1/collective never stalls the b0 pieces
                    for cc in range(2, NCC):
                        wt = wt2 if cc == 2 else load_wo(cc)
                        emit_C_piece(0, 0, cc, wt, ag0)
                        emit_C_piece(0, 1, cc, wt, ag1)
                    for cc in range(NCC):
                        wt = load_wo(cc)
                        emit_C_piece(1, 0, cc, wt, bg0)
                        emit_C_piece(1, 1, cc, wt, bg1)

    nc.finalize()
    return nc


def make_in_maps(hidden_states, position_ids, Wq, Wk, Wv, Wo):
    PKW = BS + 3 * DL + H + 640 + 640 + 256
    C_W, C_WO, C_CS = BS, BS + 3 * DL, BS + 3 * DL + H

    xT = np.ascontiguousarray(
        np.asarray(hidden_states, np.float32).reshape(BS, H).T).astype(
        ml_dtypes.bfloat16)

    inv_freq = (1.0 / (THETA ** (np.arange(0, ROT, 2, dtype=np.float32) / ROT)))
    pos = np.asarray(position_ids, np.float32).reshape(-1)     # [BS]
    ang = pos[:, None] * inv_freq[None, :]                     # [BS, 10]
    cosN = np.concatenate([np.cos(ang), np.cos(ang)], 1)       # [BS, 20]
    sinN = np.concatenate([-np.sin(ang), np.sin(ang)], 1)
    # device layout: [p, m*20+d] for global row m*128+p
    cosP = cosN.reshape(BS // 128, 128, ROT).transpose(1, 0, 2).reshape(
        128, BS // 128 * ROT)
    sinP = sinN.reshape(BS // 128, 128, ROT).transpose(1, 0, 2).reshape(
        128, BS // 128 * ROT)

    identity = np.eye(128, dtype=np.float32)
    i = np.arange(128)[:, None]
    j = np.arange(128)[None, :]
    trimask = (i <= j).astype(np.float32)

    Wq = np.asarray(Wq, np.float32)
    Wk = np.asarray(Wk, np.float32)
    Wv = np.asarray(Wv, np.float32)
    wo_bf = np.asarray(Wo, np.float32).astype(ml_dtypes.bfloat16)

    base = np.zeros((H, PKW), dtype=ml_dtypes.bfloat16)
    base[:, 0:BS] = xT
    base[:, C_WO:C_WO + H] = wo_bf
    base[0:128, C_CS:C_CS + 640] = cosP.astype(ml_dtypes.bfloat16)
    base[0:128, C_CS + 640:C_CS + 1280] = sinP.astype(ml_dtypes.bfloat16)
    base[0:128, C_CS + 1280:C_CS + 1408] = identity.astype(
        ml_dtypes.bfloat16)
    base[0:128, C_CS + 1408:C_CS + 1536] = trimask.astype(ml_dtypes.bfloat16)

    in_maps = []
    for c in range(N_CORES):
        sl = slice(c * DL, (c + 1) * DL)
        p = base.copy()
        p[:, C_W:C_W + DL] = Wq[:, sl].astype(ml_dtypes.bfloat16)
        p[:, C_W + DL:C_W + 2 * DL] = Wk[:, sl].astype(ml_dtypes.bfloat16)
        p[:, C_W + 2 * DL:C_W + 3 * DL] = Wv[:, sl].astype(ml_dtypes.bfloat16)
        in_maps.append({"pack": p})
    return in_maps


def assemble(outs):
    """outs[c] = per-core 'out' [B*2*RPH, H] -> full [B, S, H]."""
    full = np.empty((B, S, H), np.float32)
    for c, o in enumerate(outs):
        for b in range(B):
            for hf in range(2):
                r = hf * (S // 2) + c * RPH
                full[b, r:r + RPH, :] = o[(b * 2 + hf) * RPH:
                                          (b * 2 + hf + 1) * RPH]
    return full


def kernel(hidden_states, attention_mask, position_ids, Wq, Wk, Wv, Wo):
    if "nc" not in _cache:
        _cache["nc"] = build_bass()
    nc = _cache["nc"]

    in_maps = make_in_maps(hidden_states, position_ids, Wq, Wk, Wv, Wo)
    res = run_bass_kernel_spmd(nc, in_maps, list(range(N_CORES)))
    return assemble([res.results[c]["out"] for c in range(N_CORES)])



# revision 37
# speedup vs baseline: 1.0044x; 1.0044x over previous
"""Trainium2 Bass kernel for nn_Attention_46840913330813.

Full attention layer: QKV proj + partial RoPE (rot=20 of 80) + causal
softmax attention + output proj.  B=2, S=2048, H=2560, 32 heads x 80.

Sharding: tensor-parallel over heads for QKV+attention (4 heads/core on
8 cores); AllToAll collectives then redistribute the per-head attention
outputs so the output projection is row-parallel (each core owns 128
query rows per batch-half and multiplies by the FULL Wo).  Four
quarter-size AllToAlls (one per batch half, ~0.7 MB each) are launched
as soon as their half of the attention finishes, so all but the last
hide under compute.  All matmul operands bf16 (tolerance 2e-2), PSUM
accumulation f32.

Schedule (single PSUM scope; 2 banks QKV proj, 4 banks scores, 2 banks
attn-accum/out-proj; Q/K seq->d transposes are plain identity matmuls
into drained psum regions — they pipeline at ~N/f marginal cost unlike
transpose-mode or DMA-XBAR):
  W1: phase A batch 0 (chunk 0 runs kt-outer over 8 psum accumulators
      so the PE tracks the cold-start DMA front; QKV proj + RoPE +
      transpose, all kept in SBUF)
  W2: phase A batch 1 interleaved with phase B batch 0
      A2A(b0,h0) after the 8th iter, A2A(b0,h1) at the end
  W3: phase B batch 1 with phase C batch 0 pieces from iter 6 on;
      A2A(b1,h1) at the end, its ag load issued from the gpsimd queue
      (a waiting DMA on sync/scalar would block the ring in-order)
  tail: C(b0) cc2..4 cover the last collective, then C(b1).
Ring discipline: sync carries a_sb outputs (collective inputs) + bulk
loads; scalar carries tables/x_hi/odd w slices + the wt2 prefetch; exp
activations share the scalar engine so no waiting DMA may precede them.
Host reassembles 128-row slices.
"""

import math

import numpy as np
import ml_dtypes

import concourse.mybir as mybir
import concourse.tile as tile
from concourse import bacc
from concourse.bass_utils import run_bass_kernel_spmd

N_CORES = 8
B, S, H = 2, 2048, 2560
BS = B * S                      # 4096
NH, HD = 32, 80                 # heads, head dim
HL = NH // N_CORES              # 4 local heads
DL = HL * HD                    # 320 local feature width
ROT = 20                        # rotary dims
THETA = 10000.0
KT = H // 128                   # 20 contraction tiles
SCALE = 1.0 / math.sqrt(HD)
SHIFT = -5.0                    # uniform pre-exp shift (cancels in softmax)
QCH = 512                       # attention q-chunk
NQC = S // QCH                  # 4 q chunks per batch
ACH = 512                       # phase A chunk width
NCPB = S // ACH                 # 4 phase A chunks per batch
RPH = 128                      # output rows per core per batch-half
CCH = 512                       # phase C output-column chunk
NCC = H // CCH                  # 5 phase C column chunks

F32 = mybir.dt.float32
BF16 = mybir.dt.bfloat16

_cache = {}


def build_bass():
    nc = bacc.Bacc(None, target_bir_lowering=False, debug=False,
                   num_devices=N_CORES)

    # Single packed input (one dispatch arg): columns are
    #   [ xT (BS) | wall (3*DL) | wo (H) | cosP 640 | sinP 640 | idtri 256 ]
    # xT/wall/wo span all H rows; the tables live in rows 0:128 with the
    # per-128-row-tile layout the kernel loads (cosP[p, m*20+d]).
    PKW = BS + 3 * DL + H + 640 + 640 + 256
    C_W, C_WO, C_CS = BS, BS + 3 * DL, BS + 3 * DL + H
    pack = nc.declare_dram_parameter("pack", [H, PKW], BF16, isOutput=False)
    xT = pack[:, 0:BS]
    wall = pack[:, C_W:C_W + 3 * DL]
    wo = pack[:, C_WO:C_WO + H]
    out = nc.declare_dram_parameter("out", [B * 2 * RPH, H], F32,
                                    isOutput=True)

    with tile.TileContext(nc) as tc:
        with tc.tile_pool(name="dram", bufs=1, space="DRAM") as dram:
            a2a_in = [[dram.tile([N_CORES * DL, RPH], BF16,
                                 name=f"a2a_in{b}{hf}", tag=f"a2a_in{b}{hf}")
                       for hf in range(2)] for b in range(B)]
            a2a_out = [[dram.tile([N_CORES * DL, RPH], BF16,
                                  name=f"a2a_out{b}{hf}",
                                  tag=f"a2a_out{b}{hf}")
                        for hf in range(2)] for b in range(B)]

            with tc.tile_pool(name="persist", bufs=1) as persist, \
                 tc.tile_pool(name="mpool", bufs=1) as mpool, \
                 tc.tile_pool(name="epool", bufs=8) as epool, \
                 tc.tile_pool(name="apool", bufs=4) as apool, \
                 tc.tile_pool(name="nat_ps", bufs=2, space="PSUM") as nat_ps, \
                 tc.tile_pool(name="big_ps", bufs=2, space="PSUM") as big_ps, \
                 tc.tile_pool(name="sm_ps", bufs=2, space="PSUM") as sm_ps:

                # QT/KT per batch: [d, t(q/k), h, seq]; V ones-augmented.
                # Partition dim 128: rows 80:128 hold transpose padding
                # garbage that phase B never reads.
                stage = [persist.tile([128, 2, HL, S], BF16, name=f"stage{b}",
                                      tag=f"stage{b}") for b in range(B)]
                v_st = [persist.tile([128, S // 128, HL, HD + 1], BF16,
                                     name=f"v{b}", tag=f"v{b}")
                        for b in range(B)]

                idtri = mpool.tile([128, 256], BF16, name="idtri")
                shift_sb = mpool.tile([128, 1], F32, name="shift_sb")
                onesA = mpool.tile([128, 1], F32, name="onesA")

                def emit_collective(b, hf):
                    nc.gpsimd.collective_compute(
                        "AllToAll", mybir.AluOpType.bypass,
                        replica_groups=[list(range(N_CORES))],
                        ins=[a2a_in[b][hf][:]], outs=[a2a_out[b][hf][:]])

                def emit_B_iter(b, qc, h):
                    """Attention for one (batch, 512-q chunk, head)."""
                    q0 = qc * QCH
                    qap = stage[b][0:HD, 0, h, q0:q0 + QCH]
                    aps_t = sm_ps.tile([128, QCH], F32, name="aps",
                                       tag="small")
                    aps = aps_t[0:HD + 1, :]
                    nfull = qc * (QCH // 128)
                    # off-diagonal k-tiles, exp'd in pairs
                    for kp in range(nfull // 2):
                        sps = big_ps.tile([128, 2 * QCH], F32, name="sps",
                                          tag="big")
                        ex = epool.tile([128, 2 * QCH], BF16, name="ex",
                                        tag="exp")
                        for g in range(2):
                            kt = 2 * kp + g
                            nc.tensor.matmul(
                                sps[:, g * QCH:(g + 1) * QCH],
                                stage[b][0:HD, 1, h, kt * 128:(kt + 1) * 128],
                                qap, start=True, stop=True)
                        nc.scalar.activation(
                            ex[:], sps[:], mybir.ActivationFunctionType.Exp,
                            bias=shift_sb[:], scale=SCALE)
                        for g in range(2):
                            kt = 2 * kp + g
                            nc.tensor.matmul(
                                aps[:], v_st[b][:, kt, h, :],
                                ex[:, g * QCH:(g + 1) * QCH],
                                start=(kt == 0), stop=False)
                    # diagonal tiles o=0..3 at widths 512/384/256/128,
                    # grouped (0,1) and (2,3); one 128x128 triangle mask
                    for (o1, o2) in ((0, 1), (2, 3)):
                        w1, w2 = QCH - o1 * 128, QCH - o2 * 128
                        sps = big_ps.tile([128, 2 * QCH], F32, name="sps",
                                          tag="big")
                        ex = epool.tile([128, 2 * QCH], BF16, name="ex",
                                        tag="exp")
                        for (o, w, c0) in ((o1, w1, 0), (o2, w2, w1)):
                            kt = nfull + o
                            nc.tensor.matmul(
                                sps[:, c0:c0 + w],
                                stage[b][0:HD, 1, h, kt * 128:(kt + 1) * 128],
                                stage[b][0:HD, 0, h, q0 + o * 128:q0 + QCH],
                                start=True, stop=True)
                        nc.scalar.activation(
                            ex[:, 0:w1 + w2], sps[:, 0:w1 + w2],
                            mybir.ActivationFunctionType.Exp,
                            bias=shift_sb[:], scale=SCALE)
                        for (o, w, c0) in ((o1, w1, 0), (o2, w2, w1)):
                            nc.vector.tensor_mul(ex[:, c0:c0 + 128],
                                                 ex[:, c0:c0 + 128],
                                                 idtri[:, 128:256])
                            kt = nfull + o
                            nc.tensor.matmul(
                                aps[:, o * 128:QCH], v_st[b][:, kt, h, :],
                                ex[:, c0:c0 + w],
                                start=(kt == 0), stop=(o == 3))
                    rec = apool.tile([1, QCH], F32, name="rec", tag="rec")
                    nc.vector.reciprocal_approx_fast(rec[:], aps[0:1, :])
                    rb = apool.tile([HD + 1, QCH], F32, name="rb", tag="rb")
                    nc.gpsimd.partition_broadcast(rb[:], rec[:])
                    a_sb = apool.tile([HD + 1, QCH], BF16, name="a_sb",
                                      tag="a_out")
                    nc.vector.tensor_mul(a_sb[:], aps[:], rb[:])
                    hf = qc // 2
                    blk0 = (qc % 2) * 4
                    dst = a2a_in[b][hf].rearrange(
                        "(blk d) n -> d blk n", blk=8)[
                        h * HD:(h + 1) * HD, blk0:blk0 + 4, :]
                    nc.sync.dma_start(
                        dst, a_sb[1:HD + 1, :].rearrange(
                            "d (q n) -> d q n", n=RPH))

                def emit_C_piece(b, hf, cc, wt, ag):
                    """out[(b,hf) 128 rows, cc-th 512 cols]."""
                    cps_t = sm_ps.tile([128, QCH], F32, name="cps",
                                       tag="small")
                    cps = cps_t[:, 0:CCH]
                    for ft in range(KT):
                        nc.tensor.matmul(cps[:], ag[:, ft, :], wt[:, ft, :],
                                         start=(ft == 0), stop=(ft == KT - 1))
                    o_sb = opool.tile([128, CCH], F32, name="o_sb",
                                      tag="o_sb")
                    nc.vector.tensor_copy(o_sb[:], cps[:])
                    r0 = (b * 2 + hf) * RPH
                    nc.sync.dma_start(
                        out[r0:r0 + RPH, cc * CCH:(cc + 1) * CCH], o_sb[:])

                def load_ag(b, hf):
                    # sync ring: a waiting DMA on the Act ring would block
                    # the exp activations queued behind it
                    ag = agpool.tile([128, KT, RPH], BF16, name="ag",
                                     tag="ag")
                    nc.sync.dma_start(
                        ag[:], a2a_out[b][hf].rearrange("(t p) n -> p t n",
                                                        p=128))
                    return ag

                def load_wo(cc, eng=None):
                    wt = wopool.tile([128, KT, CCH], BF16, name="wt",
                                     tag="wo")
                    (eng or nc.sync).dma_start(
                        wt[:], wo.rearrange("(t p) n -> p t n",
                                            p=128)[:, :,
                                                   cc * CCH:(cc + 1) * CCH])
                    return wt

                with tc.tile_pool(name="wpool", bufs=1) as wpool, \
                     tc.tile_pool(name="xpool", bufs=4) as xpool, \
                     tc.tile_pool(name="cpool", bufs=1) as cpool, \
                     tc.tile_pool(name="sbA", bufs=3) as sbA:

                    HK = KT // 2

                    def load_x(b, ci):
                        # split head pieces so the chunk's first matmuls can
                        # start after ~0.25MB instead of the full 2.5MB
                        g0 = b * S + ci * ACH
                        csl = slice(g0, g0 + ACH)
                        lo = xT[0:HK * 128, csl].rearrange(
                            "(t p) n -> p t n", p=128)
                        hi = xT[HK * 128:H, csl].rearrange(
                            "(t p) n -> p t n", p=128)
                        x_lo = xpool.tile([128, HK, ACH], BF16, name="x_lo",
                                          tag="x")
                        nc.sync.dma_start(x_lo[:, 0:2, :], lo[:, 0:2, :])
                        nc.sync.dma_start(x_lo[:, 2:HK, :], lo[:, 2:HK, :])
                        x_hi = xpool.tile([128, HK, ACH], BF16, name="x_hi",
                                          tag="x")
                        nc.scalar.dma_start(x_hi[:, 0:2, :], hi[:, 0:2, :])
                        nc.scalar.dma_start(x_hi[:, 2:HK, :], hi[:, 2:HK, :])
                        return x_lo, x_hi

                    # DMA front, interleaved by need-time of the first pass:
                    # sync:   xlo[0:2] w0 xlo[2:10] w2 w4 .. w18
                    # scalar: idtri cos sin xhi[0:2] w1 w3 xhi[2:10] w5..w19
                    w_sb = wpool.tile([128, KT, 3 * DL], BF16, name="w_sb")
                    wview = wall.rearrange("(t p) n -> p t n", p=128)
                    lo0 = xT[0:HK * 128, 0:ACH].rearrange(
                        "(t p) n -> p t n", p=128)
                    hi0 = xT[HK * 128:H, 0:ACH].rearrange(
                        "(t p) n -> p t n", p=128)
                    x_lo0 = xpool.tile([128, HK, ACH], BF16, name="x_lo",
                                       tag="x")
                    x_hi0 = xpool.tile([128, HK, ACH], BF16, name="x_hi",
                                       tag="x")
                    nc.sync.dma_start(x_lo0[:, 0:2, :], lo0[:, 0:2, :])
                    nc.scalar.dma_start(idtri[:],
                                        pack[0:128, C_CS + 1280:C_CS + 1536])
                    cosN_sb = cpool.tile([128, BS // 128, ROT], BF16,
                                         name="cosN_sb")
                    nc.scalar.dma_start(
                        cosN_sb[:],
                        pack[0:128, C_CS:C_CS + 640].rearrange(
                            "p (m d) -> p m d", d=ROT))
                    sinN_sb = cpool.tile([128, BS // 128, ROT], BF16,
                                         name="sinN_sb")
                    nc.scalar.dma_start(
                        sinN_sb[:],
                        pack[0:128, C_CS + 640:C_CS + 1280].rearrange(
                            "p (m d) -> p m d", d=ROT))
                    nc.sync.dma_start(w_sb[:, 0, :], wview[:, 0, :])
                    nc.scalar.dma_start(x_hi0[:, 0:2, :], hi0[:, 0:2, :])
                    nc.scalar.dma_start(w_sb[:, 1, :], wview[:, 1, :])
                    nc.scalar.dma_start(w_sb[:, 3, :], wview[:, 3, :])
                    nc.sync.dma_start(x_lo0[:, 2:HK, :], lo0[:, 2:HK, :])
                    for kt in range(2, KT, 2):
                        nc.sync.dma_start(w_sb[:, kt, :], wview[:, kt, :])
                    nc.scalar.dma_start(x_hi0[:, 2:HK, :], hi0[:, 2:HK, :])
                    for kt in range(5, KT, 2):
                        nc.scalar.dma_start(w_sb[:, kt, :], wview[:, kt, :])
                    x0 = (x_lo0, x_hi0)
                    # throwaway matmuls keep the PE busy through the DMA
                    # front so the p-state ramp finishes before real work
                    warm = cpool.tile([128, 16], BF16, name="warm")
                    nc.vector.memset(warm[:], 0.0)
                    wps = nat_ps.tile([128, QCH], F32, name="wps", tag="nat")
                    for i in range(110):
                        nc.tensor.matmul(wps[0:16, 0:16], warm[:], warm[:],
                                         start=(i == 0), stop=(i == 109))
                    nc.vector.memset(shift_sb[:], SHIFT)
                    nc.vector.memset(onesA[:], 1.0)
                    for b in range(B):
                        nc.vector.tensor_copy(
                            v_st[b][:, :, :, 0:1],
                            onesA[:, :, None, None].to_broadcast(
                                (128, S // 128, HL, 1)))

                    def drain_mt(b, mtl, mtg, psA, psB, tps):
                        """psum -> qk_sb -> rope -> v_st/stage for one
                        128-row tile.  psA/psB are [128, 480] psum APs;
                        tps = (tpA, tpB) [HD, 512] psum APs for the
                        identity-matmul transposes."""
                        qk_sb = sbA.tile([128, 2, HL, 128], BF16,
                                         name="qk_sb", tag="qk")
                        nc.vector.tensor_copy(
                            qk_sb[:, 0, :, 0:HD],
                            psA[:, 0:4 * HD].rearrange(
                                "p (h d) -> p h d", d=HD))
                        nc.vector.tensor_copy(
                            qk_sb[:, 1, 0:2, 0:HD],
                            psA[:, 4 * HD:480].rearrange(
                                "p (h d) -> p h d", d=HD))
                        nc.vector.tensor_copy(
                            qk_sb[:, 1, 2:4, 0:HD],
                            psB[:, 0:2 * HD].rearrange(
                                "p (h d) -> p h d", d=HD))
                        # rope: q' = q*cos + swap(q)*sin_signed
                        rtmp = sbA.tile([128, 2, HL, ROT], BF16,
                                        name="rtmp", tag="rt")
                        half = ROT // 2
                        cosb = cosN_sb[:, mtg, None, None, :].to_broadcast(
                            (128, 2, HL, ROT))
                        sinb = sinN_sb[:, mtg, None, None, :].to_broadcast(
                            (128, 2, HL, ROT))
                        nc.vector.tensor_mul(rtmp[:, :, :, 0:half],
                                             qk_sb[:, :, :, half:ROT],
                                             sinb[:, :, :, 0:half])
                        nc.vector.tensor_mul(rtmp[:, :, :, half:ROT],
                                             qk_sb[:, :, :, 0:half],
                                             sinb[:, :, :, half:ROT])
                        nc.vector.tensor_mul(qk_sb[:, :, :, 0:ROT],
                                             qk_sb[:, :, :, 0:ROT], cosb)
                        nc.vector.tensor_add(qk_sb[:, :, :, 0:ROT],
                                             qk_sb[:, :, :, 0:ROT],
                                             rtmp[:])
                        nc.vector.tensor_copy(
                            v_st[b][:, mtl, :, 1:HD + 1],
                            psB[:, 160:480].rearrange(
                                "p (h d) -> p h d", h=HL))
                        # transposes as plain identity matmuls: they
                        # pipeline with the projection matmuls (~70ns
                        # marginal) unlike transpose-mode (~275ns flat)
                        for t in range(2):
                            tp = tps[t]
                            for h in range(HL):
                                nc.tensor.matmul(
                                    tp[:, h * 128:(h + 1) * 128],
                                    qk_sb[:, t, h, 0:HD], idtri[:, 0:128],
                                    start=True, stop=True)
                            nc.vector.tensor_copy(
                                stage[b][0:HD, t, :,
                                         mtl * 128:(mtl + 1) * 128],
                                tp[:].rearrange("d (h n) -> d h n",
                                                h=HL))

                    def emit_A_chunk(b, ci, xpre=None):
                        """QKV projection for one 512-row chunk."""
                        x_lo, x_hi = xpre if xpre else load_x(b, ci)

                        def xk(kt):
                            return (x_lo[:, kt, :] if kt < HK
                                    else x_hi[:, kt - HK, :])

                        for mt in range(ACH // 128):
                            mtl = ci * (ACH // 128) + mt     # tile in batch
                            mtg = b * (S // 128) + mtl       # global tile
                            # two 1-bank psum passes (480 cols each) keep
                            # phase A's psum rotation decoupled from B's
                            psA = nat_ps.tile([128, QCH], F32, name="psA",
                                              tag="nat")
                            for kt in range(KT):
                                nc.tensor.matmul(
                                    psA[:, 0:480],
                                    xk(kt)[:, mt * 128:(mt + 1) * 128],
                                    w_sb[:, kt, 0:480],
                                    start=(kt == 0), stop=(kt == KT - 1))
                            psB = nat_ps.tile([128, QCH], F32, name="psB",
                                              tag="nat")
                            for kt in range(KT):
                                nc.tensor.matmul(
                                    psB[:, 0:480],
                                    xk(kt)[:, mt * 128:(mt + 1) * 128],
                                    w_sb[:, kt, 480:960],
                                    start=(kt == 0), stop=(kt == KT - 1))
                            tp_t = big_ps.tile([128, 2 * QCH], F32,
                                               name="tp", tag="big")
                            drain_mt(b, mtl, mtg, psA[:, 0:480],
                                     psB[:, 0:480],
                                     (tp_t[0:HD, 0:QCH],
                                      tp_t[0:HD, QCH:2 * QCH]))

                    def emit_A_chunk0(x_lo, x_hi):
                        """Chunk (0,0): kt-outer over 8 psum accumulators so
                        the PE consumes each k-tile at DMA arrival pace
                        during the cold-start front."""
                        def xk(kt):
                            return (x_lo[:, kt, :] if kt < HK
                                    else x_hi[:, kt - HK, :])

                        big1 = big_ps.tile([128, 2 * QCH], F32, name="c0a",
                                           tag="big")
                        big2 = big_ps.tile([128, 2 * QCH], F32, name="c0b",
                                           tag="big")
                        nat1 = nat_ps.tile([128, QCH], F32, name="c0c",
                                           tag="nat")
                        nat2 = nat_ps.tile([128, QCH], F32, name="c0d",
                                           tag="nat")
                        sm1 = sm_ps.tile([128, QCH], F32, name="c0e",
                                         tag="small")
                        sm2 = sm_ps.tile([128, QCH], F32, name="c0f",
                                         tag="small")
                        psA = [big1[:, 0:480], big1[:, QCH:QCH + 480],
                               big2[:, 0:480], big2[:, QCH:QCH + 480]]
                        psB = [nat1[:, 0:480], nat2[:, 0:480],
                               sm1[:, 0:480], sm2[:, 0:480]]
                        for kt in range(KT):
                            for mt in range(4):
                                nc.tensor.matmul(
                                    psA[mt],
                                    xk(kt)[:, mt * 128:(mt + 1) * 128],
                                    w_sb[:, kt, 0:480],
                                    start=(kt == 0), stop=(kt == KT - 1))
                            for mt in range(4):
                                nc.tensor.matmul(
                                    psB[mt],
                                    xk(kt)[:, mt * 128:(mt + 1) * 128],
                                    w_sb[:, kt, 480:960],
                                    start=(kt == 0), stop=(kt == KT - 1))
                        # tp reuses each tile's own drained banks
                        tps = [(big1[0:HD, 0:QCH], nat1[0:HD, 0:QCH]),
                               (big1[0:HD, QCH:2 * QCH], nat2[0:HD, 0:QCH]),
                               (big2[0:HD, 0:QCH], sm1[0:HD, 0:QCH]),
                               (big2[0:HD, QCH:2 * QCH], sm2[0:HD, 0:QCH])]
                        for mt in range(4):
                            drain_mt(0, mt, mt, psA[mt], psB[mt], tps[mt])

                    # W1: phase A batch 0
                    emit_A_chunk0(*x0)
                    for ci in range(1, NCPB):
                        emit_A_chunk(0, ci)
                    # W2: phase A batch 1 interleaved with phase B batch 0
                    # AND the already-enabled phase B batch 1 blocks (block
                    # qc only needs A(b1) chunks 0..qc).  B blocks go BEFORE
                    # their paired A chunk so the Act engine drains the exps
                    # during the projection matmuls.
                    for ci in range(NCPB):
                        for h in range(HL):
                            emit_B_iter(0, ci, h)
                        if ci == 1:
                            emit_collective(0, 0)
                        if ci == 3:
                            emit_collective(0, 1)
                        if ci >= 1:
                            for h in range(HL):
                                emit_B_iter(1, ci - 1, h)
                            if ci == 2:
                                emit_collective(1, 0)
                        emit_A_chunk(1, ci)

                # W3: the remaining phase B batch 1 block (qc=3), then C.
                # Each wo slice is loaded ONCE (cc-outer loop over all four
                # (b,hf) groups); ag tiles prefetched right after their
                # collectives land.
                with tc.tile_pool(name="wopool", bufs=3) as wopool, \
                     tc.tile_pool(name="agpool", bufs=4) as agpool, \
                     tc.tile_pool(name="opool", bufs=2) as opool:
                    ag0, ag1 = load_ag(0, 0), load_ag(0, 1)
                    bg0 = load_ag(1, 0)         # a2a(1,0) landed in W2
                    # wt2 prefetched on the idle scalar ring so cc=2 starts
                    # right after the B loop (sync stays clear for a_sb)
                    wt2 = load_wo(2, eng=nc.scalar)
                    for h in range(HL):
                        emit_B_iter(1, 3, h)
                        if h >= 2:
                            wt = load_wo(h - 2)
                            emit_C_piece(0, 0, h - 2, wt, ag0)
                            emit_C_piece(0, 1, h - 2, wt, ag1)
                    emit_collective(1, 1)
                    # bg1 issues from the gpsimd queue right after its
                    # collective so the sync ring never blocks on the wait
                    bg1 = agpool.tile([128, KT, RPH], BF16, name="ag",
                                      tag="ag")
                    nc.gpsimd.dma_start(
                        bg1[:], a2a_out[1][1].rearrange("(t p) n -> p t n",
                                                        p=128))
                    # batch-0 pieces first (their deps are long satisfied);
                    # the batch-1 group comes last so its hoisted PE-side
                    # wait on bg1/collective never stalls the b0 pieces
                    for cc in range(2, NCC):
                        wt = wt2 if cc == 2 else load_wo(cc)
                        emit_C_piece(0, 0, cc, wt, ag0)
                        emit_C_piece(0, 1, cc, wt, ag1)
                    for cc in range(NCC):
                        wt = load_wo(cc)
                        emit_C_piece(1, 0, cc, wt, bg0)
                        emit_C_piece(1, 1, cc, wt, bg1)

    nc.finalize()
    return nc


def make_in_maps(hidden_states, position_ids, Wq, Wk, Wv, Wo):
    PKW = BS + 3 * DL + H + 640 + 640 + 256
    C_W, C_WO, C_CS = BS, BS + 3 * DL, BS + 3 * DL + H

    xT = np.ascontiguousarray(
        np.asarray(hidden_states, np.float32).reshape(BS, H).T).astype(
        ml_dtypes.bfloat16)

    inv_freq = (1.0 / (THETA ** (np.arange(0, ROT, 2, dtype=np.float32) / ROT)))
    pos = np.asarray(position_ids, np.float32).reshape(-1)     # [BS]
    ang = pos[:, None] * inv_freq[None, :]                     # [BS, 10]
    cosN = np.concatenate([np.cos(ang), np.cos(ang)], 1)       # [BS, 20]
    sinN = np.concatenate([-np.sin(ang), np.sin(ang)], 1)
    # device layout: [p, m*20+d] for global row m*128+p
    cosP = cosN.reshape(BS // 128, 128, ROT).transpose(1, 0, 2).reshape(
        128, BS // 128 * ROT)
    sinP = sinN.reshape(BS // 128, 128, ROT).transpose(1, 0, 2).reshape(
        128, BS // 128 * ROT)

    identity = np.eye(128, dtype=np.float32)
    i = np.arange(128)[:, None]
    j = np.arange(128)[None, :]
    trimask = (i <= j).astype(np.float32)

    Wq = np.asarray(Wq, np.float32)
    Wk = np.asarray(Wk, np.float32)
    Wv = np.asarray(Wv, np.float32)
    wo_bf = np.asarray(Wo, np.float32).astype(ml_dtypes.bfloat16)

    base = np.zeros((H, PKW), dtype=ml_dtypes.bfloat16)
    base[:, 0:BS] = xT
    base[:, C_WO:C_WO + H] = wo_bf
    base[0:128, C_CS:C_CS + 640] = cosP.astype(ml_dtypes.bfloat16)
    base[0:128, C_CS + 640:C_CS + 1280] = sinP.astype(ml_dtypes.bfloat16)
    base[0:128, C_CS + 1280:C_CS + 1408] = identity.astype(
        ml_dtypes.bfloat16)
    base[0:128, C_CS + 1408:C_CS + 1536] = trimask.astype(ml_dtypes.bfloat16)

    in_maps = []
    for c in range(N_CORES):
        sl = slice(c * DL, (c + 1) * DL)
        p = base.copy()
        p[:, C_W:C_W + DL] = Wq[:, sl].astype(ml_dtypes.bfloat16)
        p[:, C_W + DL:C_W + 2 * DL] = Wk[:, sl].astype(ml_dtypes.bfloat16)
        p[:, C_W + 2 * DL:C_W + 3 * DL] = Wv[:, sl].astype(ml_dtypes.bfloat16)
        in_maps.append({"pack": p})
    return in_maps


def assemble(outs):
    """outs[c] = per-core 'out' [B*2*RPH, H] -> full [B, S, H]."""
    full = np.empty((B, S, H), np.float32)
    for c, o in enumerate(outs):
        for b in range(B):
            for hf in range(2):
                r = hf * (S // 2) + c * RPH
                full[b, r:r + RPH, :] = o[(b * 2 + hf) * RPH:
                                          (b * 2 + hf + 1) * RPH]
    return full


def kernel(hidden_states, attention_mask, position_ids, Wq, Wk, Wv, Wo):
    if "nc" not in _cache:
        _cache["nc"] = build_bass()
    nc = _cache["nc"]

    in_maps = make_in_maps(hidden_states, position_ids, Wq, Wk, Wv, Wo)
    res = run_bass_kernel_spmd(nc, in_maps, list(range(N_CORES)))
    return assemble([res.results[c]["out"] for c in range(N_CORES)])

